# revision 46
# baseline (speedup 1.0000x reference)
"""Trainium2 Bass kernel for nn_Conv2dTB (BN -> ternary quantize -> 3x3 conv
-> beta box-filter scaling), data-parallel over batch on 8 NeuronCores.

Contract: kernel(**inputs) takes the FULL unsharded inputs as numpy arrays and
returns the FULL [16, 256, 56, 56] float32 output. Internally the batch dim is
split 2 images/core; BN batch statistics use an on-device AllGather + local
reduce so the normalization matches the reference's full-batch statistics.

Schedule (engine queues are in-order; issue order below is deadlock-audited):
  sync:   x loads -> bbc img0 broadcasts -> img0 stores
  gpsimd: w/const loads -> stat bounce/collective -> beta DMAs -> img1 stores
  scalar: Square stats -> weight fp16 copies -> wT PSUM copies -> sqrt ->
          Sign (img0, img1) -> Abs (img0, img1)
  vector: pad memsets -> reduce_sum stats -> gather-reduce -> scale/shift ->
          min/c2 -> beta vector ops -> osb output scaling
  PE:     wT transposes (hidden under the collective) -> conv groups with
          beta channel-sum/tridiag matmuls interleaved between row-tiles
"""

import numpy as np

# Problem shapes (hardcoded per contract).
N, C, H, W = 16, 256, 56, 56
COUT = 256
KS = 3
EPS = 1e-4
N_CORES = 8
NLOC = N // N_CORES  # images per core (2)
CB = C // 128  # channel blocks (2)
COB = COUT // 128  # cout blocks (2)
RT_ROWS = 8  # image rows per pixel tile
NT = H // RT_ROWS  # row tiles per image (7)
NPIX = RT_ROWS * W  # pixels per tile (448)
HW = H * W  # 3136
PH = H + 2  # padded rows (58)
PW = W + 2  # padded cols (58)
QR = 14  # rows per Sign chunk (4 chunks per image)
NQ = H // QR
COUNT = float(N * H * W)  # BN reduction count (full batch)

_CACHE = {}


def _build():
    import concourse.tile as tile
    from concourse import bacc, mybir
    import concourse.bass as bass

    f32 = mybir.dt.float32
    f16 = mybir.dt.float16
    AF = mybir.ActivationFunctionType
    ALU = mybir.AluOpType

    nc = bacc.Bacc("TRN2", target_bir_lowering=False, debug=False,
                   num_devices=N_CORES)

    # ---- external I/O ----
    x_d = nc.dram_tensor("x", [NLOC, C, H, W], f32, kind="ExternalInput").ap()
    gamma_d = nc.dram_tensor("bn_gamma", [C], f32, kind="ExternalInput").ap()
    bnbeta_d = nc.dram_tensor("bn_beta", [C], f32, kind="ExternalInput").ap()
    w_d = nc.dram_tensor("conv_w", [COUT, C, KS, KS], f32,
                         kind="ExternalInput").ap()
    cb_d = nc.dram_tensor("conv_b", [COUT], f32, kind="ExternalInput").ap()
    bb_d = nc.dram_tensor("beta_conv_b", [1], f32, kind="ExternalInput").ap()
    # host-provided constants
    ident_d = nc.dram_tensor("ident128", [128, 128], f32,
                             kind="ExternalInput").ap()
    t3_d = nc.dram_tensor("tridiag", [H, H], f32, kind="ExternalInput").ap()
    cnt_d = nc.dram_tensor("boxcnt", [H, W], f32, kind="ExternalInput").ap()
    out_d = nc.dram_tensor("out", [NLOC, COUT, H, W], f32,
                           kind="ExternalOutput").ap()

    with tile.TileContext(nc) as tc:
        with (
            tc.tile_pool(name="persist", bufs=1) as persist,
            tc.tile_pool(name="scratch", bufs=1) as scratch,
            tc.tile_pool(name="wtmp", bufs=2) as wtmp,
            tc.tile_pool(name="srt", bufs=2) as srt_pool,
            tc.tile_pool(name="stage", bufs=3) as stage,
            tc.tile_pool(name="outp", bufs=6) as outp,
            tc.tile_pool(name="betabc", bufs=7) as betabc,
            tc.tile_pool(name="ps_y", bufs=7, space="PSUM") as ps_y,
            tc.tile_pool(name="ps_m", bufs=1, space="PSUM") as ps_m,
            tc.tile_pool(name="dram", bufs=1, space="DRAM") as dram,
        ):
            # ================= phase 0: loads =================
            # NOTE: the CC cores that execute the stats AllReduce are not
            # ready until ~75-85us into the NEFF regardless of trigger time;
            # warmup collectives only queue in front of the real one, so the
            # real AllReduce is issued bare as soon as local stats finish.

            # x on the sync ring only (empirically fastest: splitting across
            # rings slowed the aggregate stream down)
            x_sb = persist.tile([128, NLOC, CB, HW], f32)
            for img in range(NLOC):
                xv = x_d[img].rearrange("(cb p) h w -> cb p (h w)", p=128)
                for cbk in range(CB):
                    for a in range(2):
                        sl = slice(a * (HW // 2), (a + 1) * (HW // 2))
                        nc.sync.dma_start(out=x_sb[:, img, cbk, sl],
                                          in_=xv[cbk][:, sl])

            # weights + consts on the gpsimd ring; f32 staged through a
            # small recycled pool (converted to f16 right away)
            wv = w_d.rearrange("(cob p) c k1 k2 -> cob p c (k1 k2)", p=128)
            w_chunks = {}
            for cob in range(COB):
                for a in range(2):
                    sl = slice(a * (C // 2), (a + 1) * (C // 2))
                    wt = wtmp.tile([128, C // 2, KS * KS], f32, tag="wt")
                    nc.gpsimd.dma_start(out=wt[:], in_=wv[cob][:, sl, :])
                    w_chunks[(cob, a)] = wt

            ident_sb = persist.tile([128, 128], f32)
            nc.gpsimd.dma_start(out=ident_sb[:], in_=ident_d[:])
            t3_sb = persist.tile([H, H], f32)
            nc.gpsimd.dma_start(out=t3_sb[:], in_=t3_d[:])
            cnt_sb = persist.tile([H, W], f32)
            nc.gpsimd.dma_start(out=cnt_sb[:], in_=cnt_d[:])
            gamma_sb = persist.tile([128, CB], f32)
            nc.gpsimd.dma_start(out=gamma_sb[:],
                                in_=gamma_d.rearrange("(cb p) -> p cb", p=128))
            bnbeta_sb = persist.tile([128, CB], f32)
            nc.gpsimd.dma_start(out=bnbeta_sb[:],
                                in_=bnbeta_d.rearrange("(cb p) -> p cb", p=128))
            convb_cols = persist.tile([128, COB], f32)
            nc.gpsimd.dma_start(out=convb_cols[:],
                                in_=cb_d.rearrange("(cob p) -> p cob", p=128))
            bb56 = persist.tile([H, 1], f32)
            bbsrc = bb_d[0:1]
            nc.gpsimd.dma_start(
                out=bb56[:],
                in_=bass.AP(tensor=bbsrc.tensor, offset=bbsrc.offset,
                            ap=[[0, H], [1, 1]]),
            )

            # ============ phase 0b: cheap vector init (fills DMA wait) ======
            # t_pad is stored flat (+64 slack) so conv matmuls can read one
            # contiguous 464-element window per tap: columns j with
            # j%58 in {56,57} hit the zero pad and land in discarded PSUM
            # columns, and the last tile's final taps read into the slack.
            t_flat = persist.tile([128, CB, NLOC, PH * PW + 64], f16)

            def t_view(cbk, img):
                return t_flat[:, cbk, img, 0:PH * PW].rearrange(
                    "p (h w) -> p h w", w=PW)

            for cbk in range(CB):
                for img in range(NLOC):
                    tv = t_view(cbk, img)
                    nc.vector.memset(tv[:, 0, :], 0.0)
                    nc.vector.memset(tv[:, PH - 1, :], 0.0)
                    nc.vector.memset(tv[:, 1:PH - 1, 0], 0.0)
                    nc.vector.memset(tv[:, 1:PH - 1, PW - 1], 0.0)
                    nc.vector.memset(t_flat[:, cbk, img, PH * PW:], 0.0)
            cg58 = persist.tile([H, NLOC, PW], f32)
            for img in range(NLOC):
                nc.vector.memset(cg58[:, img, 0:1], 0.0)
                nc.vector.memset(cg58[:, img, PW - 1:PW], 0.0)
            ones_c = persist.tile([128, 1], f16)
            nc.vector.memset(ones_c[:], 1.0)

            # ================ phase 1: BN statistics =================
            # per-chunk: vector reduce_sum + scalar Square-with-accum,
            # pipelined against x DMA arrivals
            sum_s = persist.tile([128, CB, NLOC, 2], f32)
            sq_s = persist.tile([128, CB, NLOC, 2], f32)
            for img in range(NLOC):
                for cbk in range(CB):
                    for a in range(2):
                        sl = slice(a * (HW // 2), (a + 1) * (HW // 2))
                        nc.vector.reduce_sum(sum_s[:, cbk, img, a:a + 1],
                                             x_sb[:, img, cbk, sl],
                                             axis=mybir.AxisListType.X)
                        sq = scratch.tile([128, HW // 2], f32, tag="sq")
                        nc.scalar.activation(
                            sq[:], x_sb[:, img, cbk, sl], AF.Square,
                            accum_out=sq_s[:, cbk, img, a:a + 1])

            # partial[:, 0:2] = per-cb sum, [:, 2:4] = per-cb sumsq
            partial = persist.tile([128, 2 * CB], f32)
            for cbk in range(CB):
                nc.vector.reduce_sum(partial[:, cbk:cbk + 1],
                                     sum_s[:, cbk], axis=mybir.AxisListType.XY)
                nc.vector.reduce_sum(partial[:, CB + cbk:CB + cbk + 1],
                                     sq_s[:, cbk], axis=mybir.AxisListType.XY)

            # 1 / (256 * boxcount + beta_conv_b) -- independent, fills gap
            den56 = persist.tile([H, W], f32)
            nc.vector.tensor_scalar(den56[:], cnt_sb[:], 256.0, bb56[:],
                                    ALU.mult, ALU.add)
            invden = persist.tile([H, W], f32)
            nc.vector.reciprocal(invden[:], den56[:])
            ident_bf = persist.tile([128, 128], f16)
            nc.vector.tensor_copy(ident_bf[:], ident_sb[:])

            # ---- w prep: f32->f16 on scalar, transposes on PE ----
            # Issued BEFORE anything that depends on the collective so the
            # scalar/PE queues run it during the collective window.
            wT = persist.tile([128, CB, KS * KS, COB, 128], f16)
            w_bf = persist.tile([128, COB, C, KS * KS], f16)
            for cob in range(COB):
                for a in range(2):
                    sl = slice(a * (C // 2), (a + 1) * (C // 2))
                    nc.scalar.copy(w_bf[:, cob, sl, :],
                                   w_chunks.pop((cob, a))[:])
            for cob in range(COB):
                for cbk in range(CB):
                    for tap in range(KS * KS):
                        wsl = w_bf[:, cob, cbk * 128:(cbk + 1) * 128, tap]
                        ps_t = ps_y.tile([128, 128], f16, tag="py")
                        nc.tensor.transpose(ps_t[:], wsl, ident_bf[:])
                        nc.scalar.copy(wT[:, cbk, tap, cob, :], ps_t[:])

            # ============ phase 2: AllReduce partial stats ========
            bounce_in = dram.tile([128, 2 * CB], f32)
            bounce_out = dram.tile([128, 2 * CB], f32, tag="bout")
            nc.gpsimd.dma_start(out=bounce_in[:], in_=partial[:])
            nc.gpsimd.collective_compute(
                "AllReduce", mybir.AluOpType.add,
                replica_groups=[list(range(N_CORES))],
                ins=[bounce_in.opt()], outs=[bounce_out.opt()],
            )
            allred = persist.tile([128, 2 * CB], f32)
            nc.gpsimd.dma_start(out=allred[:], in_=bounce_out[:])

            # ============ phase 3: scale/shift (xn = x*scale + shift) =======
            ms = persist.tile([128, 2 * CB], f32)
            nc.vector.tensor_scalar_mul(ms[:], allred[:], 1.0 / COUNT)
            mean = ms[:, 0:CB]
            var = persist.tile([128, CB], f32)
            nc.vector.tensor_mul(var[:], mean, mean)
            # var = (E[x^2] + EPS) - mean^2, fused
            nc.vector.scalar_tensor_tensor(var[:], ms[:, CB:2 * CB], EPS,
                                           var[:], ALU.add, ALU.subtract)
            rvar = persist.tile([128, CB], f32)
            nc.vector.reciprocal(rvar[:], var[:])
            rstd = persist.tile([128, CB], f32)
            nc.scalar.sqrt(rstd[:], rvar[:])
            scale = persist.tile([128, CB], f32)
            nc.vector.tensor_mul(scale[:], rstd[:], gamma_sb[:])
            shift = persist.tile([128, CB], f32)
            nc.vector.tensor_mul(shift[:], mean, scale[:])
            nc.vector.tensor_sub(shift[:], bnbeta_sb[:], shift[:])

            # ============ phase 4: ternarize + clip + conv + beta ===========
            xc_sb = persist.tile([128, CB, NLOC, HW], f16)
            c2_sb = persist.tile([128, NLOC, HW], f16)
            bflat_ds = [dram.tile([H, W], f32, tag=f"bflat{i}",
                                  name=f"bflat{i}")
                        for i in range(NLOC)]
            ov = out_d.rearrange("n (cob p) h w -> n cob p (h w)", p=128)

            py_tiles = {}

            def conv_tile(img, cob, rt):
                py = ps_y.tile([128, NPIX], f32, tag="py")
                py_tiles[(img, cob, rt)] = py
                first = True
                for cbk in range(CB):
                    for ky in range(KS):
                        for kx in range(KS):
                            rhs = t_view(cbk, img)[
                                :,
                                rt * RT_ROWS + ky:
                                rt * RT_ROWS + ky + RT_ROWS,
                                kx:kx + W]
                            last = (cbk == CB - 1 and ky == KS - 1
                                    and kx == KS - 1)
                            nc.tensor.matmul(
                                py[:],
                                wT[:, cbk, ky * KS + kx, cob, :],
                                rhs, start=first, stop=last)
                            first = False

            def osb_tile(img, cob, rt, bbc):
                py = py_tiles.pop((img, cob, rt))
                osb = outp.tile([128, NPIX], f32, tag="osb")
                nc.vector.scalar_tensor_tensor(
                    osb[:], py[:], convb_cols[:, cob:cob + 1], bbc[:],
                    ALU.add, ALU.mult)
                # alternate store queues so the final stores drain in parallel
                eng = nc.sync if rt % 2 == 0 else nc.gpsimd
                eng.dma_start(
                    out=ov[img, cob][:, rt * NPIX:(rt + 1) * NPIX],
                    in_=osb[:])

            for img in range(NLOC):
                # ---- Sign into padded t on scalar (gates the conv); the
                # first chunk is exactly the 10 rows conv rt0 consumes ----
                for q0, q1 in ((0, 10), (10, 24), (24, 40), (40, 56)):
                    rs = slice(q0, q1)
                    prs = slice(1 + q0, 1 + q1)
                    for cbk in range(CB):
                        xr = x_sb[:, img, cbk, :].rearrange(
                            "p (h w) -> p h w", w=W)
                        nc.scalar.activation(
                            t_view(cbk, img)[:, prs, 1:PW - 1],
                            xr[:, rs, :],
                            AF.Sign, bias=shift[:, cbk:cbk + 1],
                            scale=scale[:, cbk:cbk + 1])
                # ---- |xn| clipped on scalar + min/c2 on vector ----
                for half in range(2):
                    sl = slice(half * (HW // 2), (half + 1) * (HW // 2))
                    for cbk in range(CB):
                        nc.scalar.activation(xc_sb[:, cbk, img, sl],
                                             x_sb[:, img, cbk, sl], AF.Abs,
                                             bias=shift[:, cbk:cbk + 1],
                                             scale=scale[:, cbk:cbk + 1])
                for half in range(2):
                    sl = slice(half * (HW // 2), (half + 1) * (HW // 2))
                    for cbk in range(CB):
                        nc.vector.tensor_scalar_min(xc_sb[:, cbk, img, sl],
                                                    xc_sb[:, cbk, img, sl],
                                                    1.0)
                    nc.vector.tensor_add(c2_sb[:, img, sl],
                                         xc_sb[:, 0, img, sl],
                                         xc_sb[:, 1, img, sl])

            for img in range(NLOC):
                # ---- conv + interleaved beta for this image ----
                for cob in range(COB):
                    if cob == 0:
                        conv_tile(img, 0, 0)
                        conv_tile(img, 0, 1)
                        # beta channel sums: ones^T @ c2 -> [1, 448] per rt,
                        # then redistribute to [8, 56] rows (y on partitions)
                        for rt in range(NT):
                            cs = ps_m.tile([1, NPIX], f32, tag="ps")
                            nc.tensor.matmul(
                                cs[:], ones_c[:],
                                c2_sb[:, img, rt * NPIX:(rt + 1) * NPIX],
                                start=True, stop=True)
                            srt = srt_pool.tile([1, NPIX], f32, tag="srt")
                            nc.vector.tensor_copy(srt[:], cs[:])
                            nc.gpsimd.dma_start(
                                out=cg58[rt * RT_ROWS:(rt + 1) * RT_ROWS,
                                         img, 1:PW - 1],
                                in_=srt[:])
                        # x-direction box sum on the free dim
                        hsum = stage.tile([H, W], f32, tag="hsum")
                        cg = cg58[:, img, :]
                        nc.vector.tensor_add(hsum[:], cg[:, 0:W], cg[:, 1:W + 1])
                        nc.vector.tensor_add(hsum[:], hsum[:], cg[:, 2:W + 2])
                        conv_tile(img, 0, 2)
                        conv_tile(img, 0, 3)
                        conv_tile(img, 0, 4)
                        # y-direction box sum via tridiagonal matmul
                        tri = ps_m.tile([H, W], f32, tag="ps")
                        nc.tensor.matmul(tri[:], t3_sb[:], hsum[:],
                                         start=True, stop=True)
                        bmap = stage.tile([H, W], f32, tag="bmap")
                        nc.vector.tensor_scalar_add(bmap[:], tri[:], bb56[:])
                        nc.vector.tensor_mul(bmap[:], bmap[:], invden[:])
                        nc.gpsimd.dma_start(out=bflat_ds[img][:], in_=bmap[:])
                        conv_tile(img, 0, 5)
                        conv_tile(img, 0, 6)
                    else:
                        for rt in range(NT):
                            conv_tile(img, cob, rt)

                # ---- beta broadcast loads + output scaling ----
                bbc_eng = nc.sync if img == 0 else nc.gpsimd
                bbcs = []
                for rt in range(NT):
                    bsl = bflat_ds[img][rt * RT_ROWS:(rt + 1) * RT_ROWS, :]
                    bbc = betabc.tile([128, NPIX], f32, tag="bbc")
                    bbc_eng.dma_start(
                        out=bbc[:],
                        in_=bass.AP(tensor=bsl.tensor, offset=bsl.offset,
                                    ap=[[0, 128], [1, NPIX]]))
                    bbcs.append(bbc)
                for cob in range(COB):
                    for rt in range(NT):
                        osb_tile(img, cob, rt, bbcs[rt])

    nc.compile()
    return nc


def _consts():
    ident = np.eye(128, dtype=np.float32)
    t3 = np.zeros((H, H), dtype=np.float32)
    for i in range(H):
        for j in range(max(0, i - 1), min(H, i + 2)):
            t3[j, i] = 1.0
    r = np.minimum(np.arange(H), H - 1 - np.arange(H))
    edge = (r >= 1).astype(np.float32) + 2.0  # 2 on border rows, 3 inside
    cnt = np.outer(edge, edge).astype(np.float32)  # valid taps: 4/6/9
    return ident, t3, cnt


def _in_maps(inputs):
    x = np.ascontiguousarray(inputs["x"], dtype=np.float32)
    ident, t3, cnt = _consts()
    shared = {
        "bn_gamma": np.ascontiguousarray(inputs["bn_gamma"], np.float32),
        "bn_beta": np.ascontiguousarray(inputs["bn_beta"], np.float32),
        "conv_w": np.ascontiguousarray(inputs["conv_w"], np.float32),
        "conv_b": np.ascontiguousarray(inputs["conv_b"], np.float32),
        "beta_conv_b": np.ascontiguousarray(inputs["beta_conv_b"], np.float32),
        "ident128": ident, "tridiag": t3, "boxcnt": cnt,
    }
    return [
        {"x": np.ascontiguousarray(x[i * NLOC:(i + 1) * NLOC]), **shared}
        for i in range(N_CORES)
    ]


def _gather(res):
    out = np.concatenate([res.results[i]["out"] for i in range(N_CORES)],
                         axis=0)
    return out.astype(np.float32)


def kernel(**inputs):
    from concourse.bass_utils import run_bass_kernel_spmd

    if "nc" not in _CACHE:
        _CACHE["nc"] = _build()
    nc = _CACHE["nc"]
    res = run_bass_kernel_spmd(nc, _in_maps(inputs), list(range(N_CORES)))
    return _gather(res)


# revision 47
# speedup vs baseline: 1.0735x; 1.0735x over previous
"""Trainium2 Bass kernel for nn_Conv2dTB (BN -> ternary quantize -> 3x3 conv
-> beta box-filter scaling), data-parallel over batch on 8 NeuronCores.

Contract: kernel(**inputs) takes the FULL unsharded inputs as numpy arrays and
returns the FULL [16, 256, 56, 56] float32 output. Internally the batch dim is
split 2 images/core; BN batch statistics use an on-device AllGather + local
reduce so the normalization matches the reference's full-batch statistics.

Schedule (engine queues are in-order; issue order below is deadlock-audited):
  sync:   x loads -> bbc img0 broadcasts -> img0 stores
  gpsimd: w/const loads -> stat bounce/collective -> beta DMAs -> img1 stores
  scalar: Square stats -> weight fp16 copies -> wT PSUM copies -> sqrt ->
          Sign (img0, img1) -> Abs (img0, img1)
  vector: pad memsets -> reduce_sum stats -> gather-reduce -> scale/shift ->
          min/c2 -> beta vector ops -> osb output scaling
  PE:     wT transposes (hidden under the collective) -> conv groups with
          beta channel-sum/tridiag matmuls interleaved between row-tiles
"""

import numpy as np

# Problem shapes (hardcoded per contract).
N, C, H, W = 16, 256, 56, 56
COUT = 256
KS = 3
EPS = 1e-4
N_CORES = 8
NLOC = N // N_CORES  # images per core (2)
CB = C // 128  # channel blocks (2)
COB = COUT // 128  # cout blocks (2)
RT_ROWS = 8  # image rows per pixel tile
NT = H // RT_ROWS  # row tiles per image (7)
NPIX = RT_ROWS * W  # pixels per tile (448)
HW = H * W  # 3136
PH = H + 2  # padded rows (58)
PW = W + 2  # padded cols (58)
QR = 14  # rows per Sign chunk (4 chunks per image)
NQ = H // QR
COUNT = float(N * H * W)  # BN reduction count (full batch)

_CACHE = {}


def _build():
    import concourse.tile as tile
    from concourse import bacc, mybir
    import concourse.bass as bass

    f32 = mybir.dt.float32
    f16 = mybir.dt.float16
    AF = mybir.ActivationFunctionType
    ALU = mybir.AluOpType

    nc = bacc.Bacc("TRN2", target_bir_lowering=False, debug=False,
                   num_devices=N_CORES)

    # ---- external I/O ----
    x_d = nc.dram_tensor("x", [NLOC, C, H, W], f32, kind="ExternalInput").ap()
    gamma_d = nc.dram_tensor("bn_gamma", [C], f32, kind="ExternalInput").ap()
    bnbeta_d = nc.dram_tensor("bn_beta", [C], f32, kind="ExternalInput").ap()
    w_d = nc.dram_tensor("conv_w", [COUT, C, KS, KS], f32,
                         kind="ExternalInput").ap()
    cb_d = nc.dram_tensor("conv_b", [COUT], f32, kind="ExternalInput").ap()
    bb_d = nc.dram_tensor("beta_conv_b", [1], f32, kind="ExternalInput").ap()
    # host-provided constants
    ident_d = nc.dram_tensor("ident128", [128, 128], f32,
                             kind="ExternalInput").ap()
    t3_d = nc.dram_tensor("tridiag", [H, H], f32, kind="ExternalInput").ap()
    cnt_d = nc.dram_tensor("boxcnt", [H, W], f32, kind="ExternalInput").ap()
    out_d = nc.dram_tensor("out", [NLOC, COUT, H, W], f32,
                           kind="ExternalOutput").ap()

    with tile.TileContext(nc) as tc:
        with (
            tc.tile_pool(name="persist", bufs=1) as persist,
            tc.tile_pool(name="scratch", bufs=1) as scratch,
            tc.tile_pool(name="wtmp", bufs=2) as wtmp,
            tc.tile_pool(name="srt", bufs=2) as srt_pool,
            tc.tile_pool(name="stage", bufs=3) as stage,
            tc.tile_pool(name="outp", bufs=6) as outp,
            tc.tile_pool(name="betabc", bufs=7) as betabc,
            tc.tile_pool(name="ps_y", bufs=7, space="PSUM") as ps_y,
            tc.tile_pool(name="ps_m", bufs=1, space="PSUM") as ps_m,
            tc.tile_pool(name="dram", bufs=1, space="DRAM") as dram,
        ):
            # ================= phase 0: loads =================
            # NOTE: the CC cores that execute the stats AllReduce are not
            # ready until ~75-85us into the NEFF regardless of trigger time;
            # warmup collectives only queue in front of the real one, so the
            # real AllReduce is issued bare as soon as local stats finish.

            # x on the sync ring only (empirically fastest: splitting across
            # rings slowed the aggregate stream down)
            x_sb = persist.tile([128, NLOC, CB, HW], f32)
            for img in range(NLOC):
                xv = x_d[img].rearrange("(cb p) h w -> cb p (h w)", p=128)
                for cbk in range(CB):
                    for a in range(2):
                        sl = slice(a * (HW // 2), (a + 1) * (HW // 2))
                        nc.sync.dma_start(out=x_sb[:, img, cbk, sl],
                                          in_=xv[cbk][:, sl])

            # weights + consts on the gpsimd ring; f32 staged through a
            # small recycled pool (converted to f16 right away)
            wv = w_d.rearrange("(cob p) c k1 k2 -> cob p c (k1 k2)", p=128)
            w_chunks = {}
            for cob in range(COB):
                for a in range(2):
                    sl = slice(a * (C // 2), (a + 1) * (C // 2))
                    wt = wtmp.tile([128, C // 2, KS * KS], f32, tag="wt")
                    nc.gpsimd.dma_start(out=wt[:], in_=wv[cob][:, sl, :])
                    w_chunks[(cob, a)] = wt

            ident_sb = persist.tile([128, 128], f32)
            nc.gpsimd.dma_start(out=ident_sb[:], in_=ident_d[:])
            t3_sb = persist.tile([H, H], f32)
            nc.gpsimd.dma_start(out=t3_sb[:], in_=t3_d[:])
            cnt_sb = persist.tile([H, W], f32)
            nc.gpsimd.dma_start(out=cnt_sb[:], in_=cnt_d[:])
            gamma_sb = persist.tile([128, CB], f32)
            nc.gpsimd.dma_start(out=gamma_sb[:],
                                in_=gamma_d.rearrange("(cb p) -> p cb", p=128))
            bnbeta_sb = persist.tile([128, CB], f32)
            nc.gpsimd.dma_start(out=bnbeta_sb[:],
                                in_=bnbeta_d.rearrange("(cb p) -> p cb", p=128))
            convb_cols = persist.tile([128, COB], f32)
            nc.gpsimd.dma_start(out=convb_cols[:],
                                in_=cb_d.rearrange("(cob p) -> p cob", p=128))
            bb56 = persist.tile([H, 1], f32)
            bbsrc = bb_d[0:1]
            nc.gpsimd.dma_start(
                out=bb56[:],
                in_=bass.AP(tensor=bbsrc.tensor, offset=bbsrc.offset,
                            ap=[[0, H], [1, 1]]),
            )

            # ============ phase 0b: cheap vector init (fills DMA wait) ======
            # t_pad is stored flat (+64 slack) so conv matmuls can read one
            # contiguous 464-element window per tap: columns j with
            # j%58 in {56,57} hit the zero pad and land in discarded PSUM
            # columns, and the last tile's final taps read into the slack.
            t_flat = persist.tile([128, CB, NLOC, PH * PW + 64], f16)

            def t_view(cbk, img):
                return t_flat[:, cbk, img, 0:PH * PW].rearrange(
                    "p (h w) -> p h w", w=PW)

            for cbk in range(CB):
                for img in range(NLOC):
                    tv = t_view(cbk, img)
                    nc.vector.memset(tv[:, 0, :], 0.0)
                    nc.vector.memset(tv[:, PH - 1, :], 0.0)
                    nc.vector.memset(tv[:, 1:PH - 1, 0], 0.0)
                    nc.vector.memset(tv[:, 1:PH - 1, PW - 1], 0.0)
                    nc.vector.memset(t_flat[:, cbk, img, PH * PW:], 0.0)
            cg58 = persist.tile([H, NLOC, PW], f32)
            for img in range(NLOC):
                nc.vector.memset(cg58[:, img, 0:1], 0.0)
                nc.vector.memset(cg58[:, img, PW - 1:PW], 0.0)
            ones_c = persist.tile([128, 1], f16)
            nc.vector.memset(ones_c[:], 1.0)

            # ================ phase 1: BN statistics =================
            # per-chunk: vector reduce_sum + scalar Square-with-accum,
            # pipelined against x DMA arrivals
            sum_s = persist.tile([128, CB, NLOC, 2], f32)
            sq_s = persist.tile([128, CB, NLOC, 2], f32)
            for img in range(NLOC):
                for cbk in range(CB):
                    for a in range(2):
                        sl = slice(a * (HW // 2), (a + 1) * (HW // 2))
                        nc.vector.reduce_sum(sum_s[:, cbk, img, a:a + 1],
                                             x_sb[:, img, cbk, sl],
                                             axis=mybir.AxisListType.X)
                        sq = scratch.tile([128, HW // 2], f32, tag="sq")
                        nc.scalar.activation(
                            sq[:], x_sb[:, img, cbk, sl], AF.Square,
                            accum_out=sq_s[:, cbk, img, a:a + 1])

            # partial[:, 0:2] = per-cb sum, [:, 2:4] = per-cb sumsq
            partial = persist.tile([128, 2 * CB], f32)
            for cbk in range(CB):
                nc.vector.reduce_sum(partial[:, cbk:cbk + 1],
                                     sum_s[:, cbk], axis=mybir.AxisListType.XY)
                nc.vector.reduce_sum(partial[:, CB + cbk:CB + cbk + 1],
                                     sq_s[:, cbk], axis=mybir.AxisListType.XY)

            # 1 / (256 * boxcount + beta_conv_b) -- independent, fills gap
            den56 = persist.tile([H, W], f32)
            nc.vector.tensor_scalar(den56[:], cnt_sb[:], 256.0, bb56[:],
                                    ALU.mult, ALU.add)
            invden = persist.tile([H, W], f32)
            nc.vector.reciprocal(invden[:], den56[:])
            ident_bf = persist.tile([128, 128], f16)
            nc.vector.tensor_copy(ident_bf[:], ident_sb[:])

            # ---- w prep: f32->f16 on scalar, transposes on PE ----
            # Issued BEFORE anything that depends on the collective so the
            # scalar/PE queues run it during the collective window.
            wT = persist.tile([128, CB, KS * KS, COB, 128], f16)
            w_bf = persist.tile([128, COB, C, KS * KS], f16)
            for cob in range(COB):
                for a in range(2):
                    sl = slice(a * (C // 2), (a + 1) * (C // 2))
                    nc.scalar.copy(w_bf[:, cob, sl, :],
                                   w_chunks.pop((cob, a))[:])
            for cob in range(COB):
                for cbk in range(CB):
                    for tap in range(KS * KS):
                        wsl = w_bf[:, cob, cbk * 128:(cbk + 1) * 128, tap]
                        ps_t = ps_y.tile([128, 128], f16, tag="py")
                        nc.tensor.transpose(ps_t[:], wsl, ident_bf[:])
                        nc.scalar.copy(wT[:, cbk, tap, cob, :], ps_t[:])

            # ============ phase 2: AllReduce partial stats ========
            bounce_in = dram.tile([128, 2 * CB], f32)
            bounce_out = dram.tile([128, 2 * CB], f32, tag="bout")
            nc.sync.dma_start(out=bounce_in[:], in_=partial[:])
            nc.gpsimd.collective_compute(
                "AllReduce", mybir.AluOpType.add,
                replica_groups=[list(range(N_CORES))],
                ins=[bounce_in.opt()], outs=[bounce_out.opt()],
            )
            allred = persist.tile([128, 2 * CB], f32)
            nc.sync.dma_start(out=allred[:], in_=bounce_out[:])

            # ============ phase 3: scale/shift (xn = x*scale + shift) =======
            ms = persist.tile([128, 2 * CB], f32)
            nc.vector.tensor_scalar_mul(ms[:], allred[:], 1.0 / COUNT)
            mean = ms[:, 0:CB]
            var = persist.tile([128, CB], f32)
            nc.vector.tensor_mul(var[:], mean, mean)
            # var = (E[x^2] + EPS) - mean^2, fused
            nc.vector.scalar_tensor_tensor(var[:], ms[:, CB:2 * CB], EPS,
                                           var[:], ALU.add, ALU.subtract)
            rvar = persist.tile([128, CB], f32)
            nc.vector.reciprocal(rvar[:], var[:])
            rstd = persist.tile([128, CB], f32)
            nc.scalar.sqrt(rstd[:], rvar[:])
            scale = persist.tile([128, CB], f32)
            nc.vector.tensor_mul(scale[:], rstd[:], gamma_sb[:])
            shift = persist.tile([128, CB], f32)
            nc.vector.tensor_mul(shift[:], mean, scale[:])
            nc.vector.tensor_sub(shift[:], bnbeta_sb[:], shift[:])

            # ============ phase 4: ternarize + clip + conv + beta ===========
            xc_sb = persist.tile([128, CB, NLOC, HW], f16)
            c2_sb = persist.tile([128, NLOC, HW], f16)
            bflat_ds = [dram.tile([H, W], f32, tag=f"bflat{i}",
                                  name=f"bflat{i}")
                        for i in range(NLOC)]
            ov = out_d.rearrange("n (cob p) h w -> n cob p (h w)", p=128)

            py_tiles = {}

            def conv_tile(img, cob, rt):
                py = ps_y.tile([128, NPIX], f32, tag="py")
                py_tiles[(img, cob, rt)] = py
                first = True
                for cbk in range(CB):
                    for ky in range(KS):
                        for kx in range(KS):
                            rhs = t_view(cbk, img)[
                                :,
                                rt * RT_ROWS + ky:
                                rt * RT_ROWS + ky + RT_ROWS,
                                kx:kx + W]
                            last = (cbk == CB - 1 and ky == KS - 1
                                    and kx == KS - 1)
                            nc.tensor.matmul(
                                py[:],
                                wT[:, cbk, ky * KS + kx, cob, :],
                                rhs, start=first, stop=last)
                            first = False

            def osb_tile(img, cob, rt, bbc):
                py = py_tiles.pop((img, cob, rt))
                osb = outp.tile([128, NPIX], f32, tag="osb")
                nc.vector.scalar_tensor_tensor(
                    osb[:], py[:], convb_cols[:, cob:cob + 1], bbc[:],
                    ALU.add, ALU.mult)
                # alternate store queues so the final stores drain in parallel
                eng = nc.sync if rt % 2 == 0 else nc.gpsimd
                eng.dma_start(
                    out=ov[img, cob][:, rt * NPIX:(rt + 1) * NPIX],
                    in_=osb[:])

            for img in range(NLOC):
                # ---- Sign into padded t on scalar (gates the conv); the
                # first chunk is exactly the 10 rows conv rt0 consumes ----
                for q0, q1 in ((0, 10), (10, 24), (24, 40), (40, 56)):
                    rs = slice(q0, q1)
                    prs = slice(1 + q0, 1 + q1)
                    for cbk in range(CB):
                        xr = x_sb[:, img, cbk, :].rearrange(
                            "p (h w) -> p h w", w=W)
                        nc.scalar.activation(
                            t_view(cbk, img)[:, prs, 1:PW - 1],
                            xr[:, rs, :],
                            AF.Sign, bias=shift[:, cbk:cbk + 1],
                            scale=scale[:, cbk:cbk + 1])
                # ---- |xn| clipped on scalar + min/c2 on vector ----
                for half in range(2):
                    sl = slice(half * (HW // 2), (half + 1) * (HW // 2))
                    for cbk in range(CB):
                        nc.scalar.activation(xc_sb[:, cbk, img, sl],
                                             x_sb[:, img, cbk, sl], AF.Abs,
                                             bias=shift[:, cbk:cbk + 1],
                                             scale=scale[:, cbk:cbk + 1])
                for half in range(2):
                    sl = slice(half * (HW // 2), (half + 1) * (HW // 2))
                    for cbk in range(CB):
                        nc.vector.tensor_scalar_min(xc_sb[:, cbk, img, sl],
                                                    xc_sb[:, cbk, img, sl],
                                                    1.0)
                    nc.vector.tensor_add(c2_sb[:, img, sl],
                                         xc_sb[:, 0, img, sl],
                                         xc_sb[:, 1, img, sl])

            for img in range(NLOC):
                # ---- conv + interleaved beta for this image ----
                for cob in range(COB):
                    if cob == 0:
                        conv_tile(img, 0, 0)
                        conv_tile(img, 0, 1)
                        # beta channel sums: ones^T @ c2 -> [1, 448] per rt,
                        # then redistribute to [8, 56] rows (y on partitions)
                        for rt in range(NT):
                            cs = ps_m.tile([1, NPIX], f32, tag="ps")
                            nc.tensor.matmul(
                                cs[:], ones_c[:],
                                c2_sb[:, img, rt * NPIX:(rt + 1) * NPIX],
                                start=True, stop=True)
                            srt = srt_pool.tile([1, NPIX], f32, tag="srt")
                            nc.vector.tensor_copy(srt[:], cs[:])
                            nc.gpsimd.dma_start(
                                out=cg58[rt * RT_ROWS:(rt + 1) * RT_ROWS,
                                         img, 1:PW - 1],
                                in_=srt[:])
                        # x-direction box sum on the free dim
                        hsum = stage.tile([H, W], f32, tag="hsum")
                        cg = cg58[:, img, :]
                        nc.vector.tensor_add(hsum[:], cg[:, 0:W], cg[:, 1:W + 1])
                        nc.vector.tensor_add(hsum[:], hsum[:], cg[:, 2:W + 2])
                        conv_tile(img, 0, 2)
                        conv_tile(img, 0, 3)
                        conv_tile(img, 0, 4)
                        # y-direction box sum via tridiagonal matmul
                        tri = ps_m.tile([H, W], f32, tag="ps")
                        nc.tensor.matmul(tri[:], t3_sb[:], hsum[:],
                                         start=True, stop=True)
                        bmap = stage.tile([H, W], f32, tag="bmap")
                        nc.vector.tensor_scalar_add(bmap[:], tri[:], bb56[:])
                        nc.vector.tensor_mul(bmap[:], bmap[:], invden[:])
                        nc.gpsimd.dma_start(out=bflat_ds[img][:], in_=bmap[:])
                        conv_tile(img, 0, 5)
                        conv_tile(img, 0, 6)
                    else:
                        for rt in range(NT):
                            conv_tile(img, cob, rt)

                # ---- beta broadcast loads + output scaling ----
                bbc_eng = nc.sync if img == 0 else nc.gpsimd
                bbcs = []
                for rt in range(NT):
                    bsl = bflat_ds[img][rt * RT_ROWS:(rt + 1) * RT_ROWS, :]
                    bbc = betabc.tile([128, NPIX], f32, tag="bbc")
                    bbc_eng.dma_start(
                        out=bbc[:],
                        in_=bass.AP(tensor=bsl.tensor, offset=bsl.offset,
                                    ap=[[0, 128], [1, NPIX]]))
                    bbcs.append(bbc)
                for cob in range(COB):
                    for rt in range(NT):
                        osb_tile(img, cob, rt, bbcs[rt])

    nc.compile()
    return nc


def _consts():
    ident = np.eye(128, dtype=np.float32)
    t3 = np.zeros((H, H), dtype=np.float32)
    for i in range(H):
        for j in range(max(0, i - 1), min(H, i + 2)):
            t3[j, i] = 1.0
    r = np.minimum(np.arange(H), H - 1 - np.arange(H))
    edge = (r >= 1).astype(np.float32) + 2.0  # 2 on border rows, 3 inside
    cnt = np.outer(edge, edge).astype(np.float32)  # valid taps: 4/6/9
    return ident, t3, cnt


def _in_maps(inputs):
    x = np.ascontiguousarray(inputs["x"], dtype=np.float32)
    ident, t3, cnt = _consts()
    shared = {
        "bn_gamma": np.ascontiguousarray(inputs["bn_gamma"], np.float32),
        "bn_beta": np.ascontiguousarray(inputs["bn_beta"], np.float32),
        "conv_w": np.ascontiguousarray(inputs["conv_w"], np.float32),
        "conv_b": np.ascontiguousarray(inputs["conv_b"], np.float32),
        "beta_conv_b": np.ascontiguousarray(inputs["beta_conv_b"], np.float32),
        "ident128": ident, "tridiag": t3, "boxcnt": cnt,
    }
    return [
        {"x": np.ascontiguousarray(x[i * NLOC:(i + 1) * NLOC]), **shared}
        for i in range(N_CORES)
    ]


def _gather(res):
    out = np.concatenate([res.results[i]["out"] for i in range(N_CORES)],
                         axis=0)
    return out.astype(np.float32)


def kernel(**inputs):
    from concourse.bass_utils import run_bass_kernel_spmd

    if "nc" not in _CACHE:
        _CACHE["nc"] = _build()
    nc = _CACHE["nc"]
    res = run_bass_kernel_spmd(nc, _in_maps(inputs), list(range(N_CORES)))
    return _gather(res)
